# revision 35
# baseline (speedup 1.0000x reference)
"""Trainium2 Bass kernel for: out = SCALE * x @ weight.sum(axis=0).

Column-sharded over 8 cores (stripe of 512 cols each); every core computes
partial dot products for ALL 16384 batch rows over its stripe; host sums the
8 partials. All device inputs bf16 (host casts during sharding; tolerance is
2e-2 so bf16 inputs with fp32 accumulation are well within budget).

Per-core pipeline:
  Phase W (wsum = colsum of the weight stripe):
    - w_nat [W_PE, 512] natural rows-share, paired-block DMAs so PE
      ones-matmuls start with the first arrival: row partial [1,512] in PSUM
    - w_t [512, W_T] transposed rows-share split by column chunk:
      ACT accum_out (chunks 0,2) + DVE tensor_reduce (chunks 1,3)
      -> column partials [128, 4] f32
    - fold via PE transpose-matmuls (identity input): col->row, DVE add ->
      wrow; ones-matmul broadcast -> wsumB (all partitions); 4 transposes ->
      wcolT bf16 (PE lhsT)
  Phase X:
    - PE: transposed x windows, 4 chunk-matmuls per 512-batch group into
      PSUM partition-slots; two dedicated PSUM tiles (12 groups each) so
      evictions never false-conflict with later matmuls; ACT evicts with
      SCALE using 4-partition strided reads (cheap, free-size driven)
    - DVE: natural x, fused scalar_tensor_tensor per 128-row block
  DMA: three plain-DMA queues (sync/scalar HWDGE, gpsimd SWDGE), each
  ~330-350 B/ns, transfers occupying their issuing queue; windows are
  spread across queues so PE receives a steady interleaved feed.
"""

import numpy as np
import ml_dtypes

from concourse import bacc, bass, tile
import concourse.mybir as mybir
from concourse.bass_utils import run_bass_kernel_spmd

BF16NP = ml_dtypes.bfloat16

N_CORES = 8
BATCH = 16384
IN_SIZE = 4096
CS = IN_SIZE // N_CORES  # 512
W_ROWS = 4096
SCALE = 0.5
P = 128

# --- tunables ---------------------------------------------------------------
B_PE = 12288            # 24 PSUM groups of 512
B_DVE = BATCH - B_PE    # 4096 -> 32 DVE blocks
W_PE = 3072             # 24 natural blocks (12 pair-DMAs)
W_T = W_ROWS - W_PE     # 1024 transposed rows
N_DBLK = B_DVE // P     # 32
N_DTILE = N_DBLK // 4   # 8 tiles [128, 4, 512]

# x_pe windows: (cols, queue); emission below interleaves expected arrivals
XPE_WINDOWS = [
    (2048, "pool"),    # xw0
    (2048, "sync"),    # xwC
    (2048, "scalar"),  # xwB
    (2048, "pool"),    # xw1
    (2048, "scalar"),  # xwE
    (2048, "pool"),    # xw2
]
assert sum(w for w, _ in XPE_WINDOWS) == B_PE

bf16 = mybir.dt.bfloat16
fp32 = mybir.dt.float32
fp32r = mybir.dt.float32r


def build_nc(for_sim: bool = False):
    if for_sim:
        nc = bacc.Bacc(None, target_bir_lowering=False, debug=True, num_devices=1)
    else:
        nc = bacc.Bacc(None, num_devices=N_CORES)

    x_pe_t = nc.declare_dram_parameter("x_pe_t", [CS, B_PE], bf16, isOutput=False)
    x_dve = nc.declare_dram_parameter("x_dve", [B_DVE, CS], bf16, isOutput=False)
    w_nat = nc.declare_dram_parameter("w_nat", [W_PE, CS], bf16, isOutput=False)
    w_t = nc.declare_dram_parameter("w_t", [CS, W_T], bf16, isOutput=False)
    ident_ext = nc.declare_dram_parameter("ident", [P, P], fp32, isOutput=False)
    out_pe = nc.declare_dram_parameter("out_pe", [24, 512], fp32, isOutput=True)
    out_dve = nc.declare_dram_parameter("out_dve", [P, N_DBLK], fp32, isOutput=True)

    with tile.TileContext(nc) as tc:
        with (
            tc.tile_pool(name="xw", bufs=6) as xw_pool,
            tc.tile_pool(name="xd", bufs=8) as xd_pool,
            tc.tile_pool(name="wpool", bufs=1) as wpool,
            tc.tile_pool(name="aux", bufs=1) as aux,
            tc.tile_pool(name="psum", bufs=1, space="PSUM") as psum,
        ):
            qmap = {"sync": nc.sync, "scalar": nc.scalar, "pool": nc.gpsimd}

            # --- Phase W DMAs ----------------------------------------------
            # first natural pairs (PE must start ramping immediately), with
            # the transposed chunks (ACT: c0,c2 / DVE: c1,c3) interleaved
            wb_tiles = [None] * (W_PE // (2 * P))
            wt_tiles = [None] * 4

            def wb_dma(pr, q):
                wb = wpool.tile([P, 2, CS], bf16, tag=f"wb{pr}")
                qmap[q].dma_start(
                    out=wb[:],
                    in_=w_nat[pr * 2 * P:(pr + 1) * 2 * P, :].rearrange(
                        "(g p) f -> p g f", p=P))
                wb_tiles[pr] = wb

            def wt_dma(c, q):
                wt_c = wpool.tile([P, W_T], bf16, tag=f"wt{c}")
                qmap[q].dma_start(out=wt_c[:], in_=w_t[c * P:(c + 1) * P, :])
                wt_tiles[c] = wt_c

            wb_dma(0, "sync"); wb_dma(1, "scalar"); wb_dma(2, "pool")
            wt_dma(1, "pool"); wt_dma(0, "sync"); wt_dma(2, "scalar")
            wb_dma(3, "sync"); wb_dma(4, "scalar"); wb_dma(5, "pool")
            wt_dma(3, "pool")
            ident = aux.tile([P, P], fp32)
            nc.scalar.dma_start(out=ident[:], in_=ident_ext[:, :])
            wb_dma(6, "sync"); wb_dma(7, "scalar"); wb_dma(8, "pool")
            wb_dma(9, "sync"); wb_dma(10, "scalar"); wb_dma(11, "sync")

            # --- Phase W compute -------------------------------------------
            ones = aux.tile([P, 1], bf16)
            nc.vector.memset(ones[:], 1.0)
            onesr = aux.tile([1, P], fp32)
            nc.vector.memset(onesr[:], 1.0)

            # five PSUM tiles (dep tracking is tile-granular; an eviction
            # read would block later matmul writes to the same tile, so each
            # tile's evictions happen only after its last group). The final
            # single-bank tiles keep the end-of-kernel eviction tail short.
            # Groups: psT[0] e0-5, psT[1] e6-11, psT[2] e12-17 (+W scratch),
            # psT[3] e18-20, psT[4] e21-23.
            psT = []
            for ti_ in range(5):
                ps_t = psum.tile([P, 1024 if ti_ < 3 else 512], fp32,
                                 tag=f"ps{ti_}")
                psT.append(ps_t)
            psB1 = psT[2]

            n_wb = W_PE // P
            for b in range(n_wb):
                nc.tensor.matmul(
                    psB1[0:1, 0:CS], ones[:], wb_tiles[b // 2][:, b % 2, :],
                    start=(b == 0), stop=(b == n_wb - 1))

            wcol = aux.tile([P, 4], fp32)
            for c in (0, 2):  # ACT chunks
                nc.scalar.activation(
                    out=wt_tiles[c][:], in_=wt_tiles[c][:],
                    func=mybir.ActivationFunctionType.Copy,
                    accum_out=wcol[:, c:c + 1])
            for c in (1, 3):  # DVE chunks
                nc.vector.tensor_reduce(
                    out=wcol[:, c:c + 1], in_=wt_tiles[c][:],
                    axis=mybir.AxisListType.X, op=mybir.AluOpType.add)

            # fold
            for c in range(4):
                nc.tensor.matmul(
                    psB1[0:1, 512 + c * P: 512 + (c + 1) * P],
                    wcol[:, c:c + 1], ident[:, :],
                    is_transpose=True, start=True, stop=True)
            trow = aux.tile([1, CS], fp32)
            nc.vector.tensor_copy(trow[:], psB1[0:1, 512:1024])
            wrow = aux.tile([1, CS], fp32)
            nc.vector.tensor_tensor(
                out=wrow[:], in0=trow[:], in1=psB1[0:1, 0:CS],
                op=mybir.AluOpType.add)
            # lhsT form first (PE x-start is the critical path), then the
            # broadcast for the DVE share
            for c in range(4):
                nc.tensor.matmul(
                    psB1[0:P, 512 + c: 512 + c + 1],
                    wrow[0:1, c * P:(c + 1) * P], ident[0:1, 0:1],
                    is_transpose=True, start=True, stop=True)
            wcolT_bf = aux.tile([P, 4], bf16)
            nc.vector.tensor_copy(wcolT_bf[:], psB1[0:P, 512:516])
            nc.tensor.matmul(psB1[0:P, 0:512], onesr[:], wrow[:], start=True, stop=True)
            wsumB = aux.tile([P, CS], fp32)
            nc.vector.tensor_copy(wsumB[:], psB1[0:P, 0:512])

            # --- Phase X DMAs ----------------------------------------------
            xd_tiles = [None] * N_DTILE

            def xd_dma(t, eng):
                xd = xd_pool.tile([P, 4, CS], bf16, tag="xd")
                eng.dma_start(
                    out=xd[:],
                    in_=x_dve[t * 4 * P:(t + 1) * 4 * P, :].rearrange(
                        "(g p) f -> p g f", p=P))
                xd_tiles[t] = xd

            xd_dma(0, nc.sync)
            xw_tiles = []
            off = 0
            for wlen, q in XPE_WINDOWS:
                xt = xw_pool.tile([P, 4, wlen], bf16, tag="xw")
                qmap[q].dma_start(
                    out=xt[:],
                    in_=x_pe_t[:, off:off + wlen].rearrange(
                        "(c p) f -> p c f", p=P))
                xw_tiles.append((xt, wlen))
                off += wlen

            for t in range(1, N_DTILE):
                xd_dma(t, nc.gpsimd if t == N_DTILE - 1 else nc.sync)

            # --- Phase X compute -------------------------------------------
            osb_pe = aux.tile([P, 4096], fp32)
            osb_dve = aux.tile([P, N_DBLK], fp32)

            dve_t = 0

            def emit_dve_tile():
                nonlocal dve_t
                if dve_t >= N_DTILE:
                    return
                xd = xd_tiles[dve_t]
                for gi in range(4):
                    col = dve_t * 4 + gi
                    nc.vector.scalar_tensor_tensor(
                        out=xd[:, gi, :], in0=xd[:, gi, :], scalar=SCALE,
                        in1=wsumB[:],
                        op0=mybir.AluOpType.mult, op1=mybir.AluOpType.mult,
                        accum_out=osb_dve[:, col:col + 1])
                dve_t += 1

            def evict(tile_i, s_, span, col0, eng):
                if eng == "act":
                    nc.scalar.activation(
                        out=osb_pe[s_ * 32:s_ * 32 + 1, col0:col0 + span],
                        in_=psT[tile_i][s_ * 32:s_ * 32 + 1, 0:span],
                        func=mybir.ActivationFunctionType.Copy, scale=SCALE)
                else:
                    veng = nc.vector if eng == "dve" else nc.gpsimd
                    veng.tensor_scalar(
                        out=osb_pe[s_ * 32:s_ * 32 + 1, col0:col0 + span],
                        in0=psT[tile_i][s_ * 32:s_ * 32 + 1, 0:span],
                        scalar1=SCALE, scalar2=None,
                        op0=mybir.AluOpType.mult)

            e = 0
            for xt, wlen in xw_tiles:
                for s in range(wlen // 512):
                    if e < 18:
                        ti, eb = e // 6, e % 6
                        bank, slot = eb % 2, eb // 2
                    elif e < 21:
                        ti, bank, slot = 3, 0, e - 18
                    else:
                        ti, bank, slot = 4, 0, e - 21
                    ps = psT[ti][slot * 32: slot * 32 + 1,
                                 bank * 512:(bank + 1) * 512]
                    for c in range(4):
                        nc.tensor.matmul(
                            ps, wcolT_bf[:, c:c + 1],
                            xt[:, c, s * 512:(s + 1) * 512],
                            start=(c == 0), stop=(c == 3))
                    # per-tile bunched evictions after the tile's last group
                    if e in (5, 11, 17):
                        ti_done = e // 6
                        for s_ in range(3):
                            evict(ti_done, s_, 1024, ti_done * 1024, "act")
                    elif e == 20:
                        for s_ in range(3):
                            evict(3, s_, 512, 3072, "act")
                    elif e == 23:
                        # parallel final evictions on three engines
                        evict(4, 0, 512, 3584, "act")
                        evict(4, 1, 512, 3584, "dve")
                        evict(4, 2, 512, 3584, "dve")
                    e += 1
                    if e % 3 == 0:
                        emit_dve_tile()
            while dve_t < N_DTILE:
                emit_dve_tile()

            nc.gpsimd.dma_start(
                out=out_pe[0:6, :], in_=osb_pe[0:65:32, 0:1024])
            nc.sync.dma_start(
                out=out_pe[6:12, :], in_=osb_pe[0:65:32, 1024:2048])
            nc.scalar.dma_start(
                out=out_pe[12:18, :], in_=osb_pe[0:65:32, 2048:3072])
            nc.gpsimd.dma_start(
                out=out_pe[18:24, :], in_=osb_pe[0:65:32, 3072:4096])
            nc.gpsimd.dma_start(out=out_dve[:], in_=osb_dve[:])

    return nc


_NC_CACHE: dict = {}


def _get_nc():
    if "nc" not in _NC_CACHE:
        nc = build_nc()
        nc.finalize()
        _NC_CACHE["nc"] = nc
    return _NC_CACHE["nc"]


def make_in_maps(x: np.ndarray, weight: np.ndarray):
    ident = np.eye(P, dtype=np.float32)
    maps = []
    for c in range(N_CORES):
        sl = slice(c * CS, (c + 1) * CS)
        xs = x[:, sl].astype(BF16NP)
        ws = weight[:, sl].astype(BF16NP)
        maps.append({
            "x_pe_t": np.ascontiguousarray(xs[:B_PE].T),
            "x_dve": np.ascontiguousarray(xs[B_PE:]),
            "w_nat": np.ascontiguousarray(ws[:W_PE]),
            "w_t": np.ascontiguousarray(ws[W_PE:].T),
            "ident": ident,
        })
    return maps


def pe_batch_order(ope_flat: np.ndarray) -> np.ndarray:
    """out_pe [24,512] via 4 col-pieces of osb_pe[{0,32,64}, 0:4096]:
    row = piece*6 + slot*2 + sub; osb col c = piece*1024 + sub*512;
    c<3072: tile=c//1024, bank=(c%1024)//512, e=tile*6+slot*2+bank;
    c in [3072,3584): e=18+slot; c>=3584: e=21+slot."""
    out = np.empty(B_PE, dtype=ope_flat.dtype)
    ope = ope_flat.reshape(4, 3, 2, 512)
    for p in range(4):
        for s in range(3):
            for sub in range(2):
                c = p * 1024 + sub * 512
                if c < 3072:
                    e = (c // 1024) * 6 + s * 2 + (c % 1024) // 512
                elif c < 3584:
                    e = 18 + s
                else:
                    e = 21 + s
                out[e * 512:(e + 1) * 512] = ope[p, s, sub]
    return out


def assemble(results) -> np.ndarray:
    out = np.zeros(BATCH, dtype=np.float64)
    for cid in range(N_CORES):
        ope = np.asarray(results[cid]["out_pe"], dtype=np.float64)
        odv = np.asarray(results[cid]["out_dve"], dtype=np.float64)
        out[:B_PE] += pe_batch_order(ope)
        out[B_PE:] += odv.T.reshape(-1)
    return out.astype(np.float32)


def kernel(x: np.ndarray, weight: np.ndarray) -> np.ndarray:
    x = np.asarray(x, dtype=np.float32)
    weight = np.asarray(weight, dtype=np.float32)
    assert x.shape == (BATCH, IN_SIZE) and weight.shape == (W_ROWS, IN_SIZE)
    nc = _get_nc()
    res = run_bass_kernel_spmd(nc, make_in_maps(x, weight), list(range(N_CORES))).results
    return assemble(res)
